# revision 10
# baseline (speedup 1.0000x reference)
"""Trainium2 Bass kernel for 3x3 VALID conv: x[32,128,64,64] * w[256,128,3,3] + bias.

Strategy (v4): 1D Winograd F(2,3) along the width axis.
  - Data-parallel over batch: 8 cores x 4 images each; weights/bias replicated.
  - Host precomputes the input transform V = B^T d per 4-pixel tile (stride 2):
        V0 = x[2t] - x[2t+2];  V1 = x[2t+1] + x[2t+2]
        V2 = x[2t+2] - x[2t+1];  V3 = x[2t+1] - x[2t+3]
    stored [c, xi, row*31] bf16 (t-major, rows adjacent -> contiguous moving
    operands), and the weight transform Gw (per vertical tap u):
        Gw0 = w0;  Gw1 = (w0+w1+w2)/2;  Gw2 = (w0-w1+w2)/2;  Gw3 = w2
  - Device: for each (img, 16-row chunk, half, xi): accumulate 3 matmuls
    (vertical taps) into psum:  M[xi] = sum_u Gw[u,xi]^T @ V[xi, rows+u]
    -> 12 matmuls per chunk-half instead of direct conv's 18 column-streams.
  - Output transform A^T (y_even = M0+M1+M2+b, y_odd = M1-M2-M3+b) is fused
    into PSUM evacuation: ScalarE Identity-copies M1, M2, (M3-b) to bf16;
    VectorE does one fused (M0+b)+C1 scalar_tensor_tensor plus two bf16 2x
    tensor_tensor ops. Even/odd parity blocks stay contiguous; host interleaves.
  - y returned bf16-packed as [b, half, o, parity, i, t]; host upcasts and
    reshapes to [B, 256, 62, 62].
"""

import numpy as np
import ml_dtypes

import concourse.bacc as bacc
import concourse.tile as tile
from concourse import mybir
from concourse.bass_utils import run_bass_kernel_spmd

N_CORES = 8
B_FULL, C_IN, H, W = 32, 128, 64, 64
C_OUT, KH, KW = 256, 3, 3
B_LOC = B_FULL // N_CORES          # images per core
H_OUT, W_OUT = H - KH + 1, W - KW + 1   # 62, 62
N_HALF = C_OUT // 128              # 2 output-channel halves
N_XI = 4                           # F(2,3) m-points
N_T = W_OUT // 2                   # 31 winograd tiles per row
ROWS_PER_CHUNK = 16                # 16 rows x 31 tiles = 496 <= one PSUM bank
CHUNKS = [(0, 16), (16, 16), (32, 16), (48, 14)]
# V row-ranges per DMA piece: chunk c needs V rows [i0, i0+r+1]
V_PIECES = [(0, 18), (18, 34), (34, 50), (50, 64)]

_cached = {}


def _build_nc():
    f32 = mybir.dt.float32
    bf16 = mybir.dt.bfloat16
    add = mybir.AluOpType.add
    sub = mybir.AluOpType.subtract
    ident = mybir.ActivationFunctionType.Identity
    nc = bacc.Bacc()

    v_d = nc.declare_dram_parameter("v", [B_LOC, C_IN, N_XI, H * N_T], bf16,
                                    isOutput=False)
    gw_d = nc.declare_dram_parameter("gw", [C_IN, N_HALF, N_XI, KH, 128], bf16,
                                     isOutput=False)
    # bias columns: [b_h0, b_h1, -b_h0, -b_h1]
    b_d = nc.declare_dram_parameter("bias_in", [128, 2 * N_HALF], f32,
                                    isOutput=False)
    y_d = nc.declare_dram_parameter(
        "y", [B_LOC, N_HALF, 128, 2, H_OUT * N_T], bf16, isOutput=True
    )

    with tile.TileContext(nc) as tc:
        with (
            tc.tile_pool(name="const", bufs=1) as cpool,
            tc.tile_pool(name="vin", bufs=2) as vpool,
            tc.tile_pool(name="mid", bufs=3) as mpool,
            tc.tile_pool(name="out", bufs=4) as opool,
            tc.tile_pool(name="psum", bufs=8, space="PSUM") as ppool,
        ):
            gw_t = cpool.tile([C_IN, N_HALF, N_XI, KH, 128], bf16)
            b_t = cpool.tile([128, 2 * N_HALF], f32)
            warm_w = cpool.tile([128, 128], bf16)

            # PE warmup: fill the DMA head with matmuls on scratch data so the
            # HAM clock gate flips to 8/8 before the real matmuls arrive.
            nc.gpsimd.memset(warm_w[:], 0)
            warm_ps = ppool.tile([128, ROWS_PER_CHUNK * N_T], f32, tag="ps",
                                 name="warm_ps")
            N_WARM = 34
            for k in range(N_WARM):
                nc.tensor.matmul(
                    warm_ps[:, 0:128], warm_w[:], warm_w[:],
                    start=(k == 0), stop=(k == N_WARM - 1),
                )

            # Weights + bias on the ScalarE HWDGE ring, xi=0 of half 0 first
            # (unblocks the first matmul group) in parallel with V piece 0 on
            # the Sync ring.
            nc.scalar.dma_start(b_t[:], b_d[:])
            nc.scalar.dma_start(gw_t[:, 0, 0], gw_d[:, 0, 0])
            nc.scalar.dma_start(gw_t[:, 0, 1:N_XI], gw_d[:, 0, 1:N_XI])
            nc.scalar.dma_start(gw_t[:, 1], gw_d[:, 1])

            def load_v(b, split_first):
                v_t = vpool.tile([C_IN, N_XI, H * N_T], bf16, tag="v")
                pieces = V_PIECES[1:] if split_first else V_PIECES
                if split_first:
                    # First image: one DMA per xi for rows 0:18, split across
                    # the Sync + ScalarE HWDGE rings so the transfers overlap
                    # and the first matmul group starts as soon as xi=0 lands.
                    r0, r1 = V_PIECES[0]
                    for xi in range(N_XI):
                        ring = nc.sync if xi < 2 else nc.scalar
                        ring.dma_start(
                            v_t[:, xi, r0 * N_T : r1 * N_T],
                            v_d[b, :, xi, r0 * N_T : r1 * N_T],
                        )
                for k, (r0, r1) in enumerate(pieces):
                    ring = nc.scalar if (split_first and k == 1) else nc.sync
                    ring.dma_start(
                        v_t[:, :, r0 * N_T : r1 * N_T],
                        v_d[b, :, :, r0 * N_T : r1 * N_T],
                    )
                return v_t

            def mm_group(ps_tiles, v_t, i0, n, half, xi):
                t = ppool.tile([128, ROWS_PER_CHUNK * N_T], f32, tag="ps",
                               name=f"ps_{half}_{xi}")
                ps_tiles[half][xi] = t
                for u in range(KH):
                    lo = (i0 + u) * N_T
                    nc.tensor.matmul(
                        t[:, 0:n],
                        gw_t[:, half, xi, u, :],
                        v_t[:, xi, lo : lo + n],
                        start=(u == 0),
                        stop=(u == KH - 1),
                    )

            def evac(ps, b, i0, n, half, split_dma):
                # Output transform A^T + bias, fused into evacuation.
                c1 = mpool.tile([128, ROWS_PER_CHUNK * N_T], bf16, tag="c1")
                c2 = mpool.tile([128, ROWS_PER_CHUNK * N_T], bf16, tag="c2")
                c3 = mpool.tile([128, ROWS_PER_CHUNK * N_T], bf16, tag="c3")
                te = mpool.tile([128, ROWS_PER_CHUNK * N_T], bf16, tag="te")
                td = mpool.tile([128, ROWS_PER_CHUNK * N_T], bf16, tag="td")
                o_t = opool.tile([128, 2, ROWS_PER_CHUNK * N_T], bf16, tag="o")
                nc.scalar.activation(c1[:, 0:n], ps[1][:, 0:n], ident)
                # te = (M0 + b) + C1 ; y_even = te + C2
                nc.vector.scalar_tensor_tensor(
                    te[:, 0:n], ps[0][:, 0:n],
                    b_t[:, half : half + 1], c1[:, 0:n], add, add,
                )
                nc.scalar.activation(c2[:, 0:n], ps[2][:, 0:n], ident)
                # c3 = M3 - b  (bias column 2+half holds -b)
                nc.scalar.activation(
                    c3[:, 0:n], ps[3][:, 0:n], ident,
                    bias=b_t[:, 2 + half : 3 + half],
                )
                nc.vector.tensor_add(o_t[:, 0, 0:n], te[:, 0:n], c2[:, 0:n])
                y_dst = y_d[b, half, :, :, i0 * N_T : i0 * N_T + n]
                # half 0 on the Sync HWDGE ring, half 1 on the (otherwise
                # idle) GpSimd SWDGE ring: outputs drain in parallel.
                ring = nc.sync if half == 0 else nc.gpsimd
                if split_dma:
                    ring.dma_start(y_dst[:, 0], o_t[:, 0, 0:n])
                # y_odd = (C1 - C2) - C3
                nc.vector.tensor_sub(td[:, 0:n], c1[:, 0:n], c2[:, 0:n])
                nc.vector.tensor_sub(o_t[:, 1, 0:n], td[:, 0:n], c3[:, 0:n])
                if split_dma:
                    ring.dma_start(y_dst[:, 1], o_t[:, 1, 0:n])
                else:
                    ring.dma_start(y_dst, o_t[:, :, 0:n])

            for b in range(B_LOC):
                v_t = load_v(b, split_first=(b == 0))
                for ci, (i0, r) in enumerate(CHUNKS):
                    n = r * N_T
                    ps_tiles = [[None] * N_XI for _ in range(N_HALF)]
                    last = b == B_LOC - 1 and ci == len(CHUNKS) - 1
                    if b == 0 and ci == 0:
                        # Cold start: interleave halves so each V xi-piece
                        # feeds two matmul groups back to back.
                        for xi in range(N_XI):
                            for half in range(N_HALF):
                                mm_group(ps_tiles, v_t, i0, n, half, xi)
                        for half in range(N_HALF):
                            evac(ps_tiles[half], b, i0, n, half, split_dma=False)
                    else:
                        for half in range(N_HALF):
                            for xi in range(N_XI):
                                mm_group(ps_tiles, v_t, i0, n, half, xi)
                            evac(ps_tiles[half], b, i0, n, half,
                                 split_dma=(last and half == N_HALF - 1))

    nc.compile()
    if not nc.is_finalized():
        nc.finalize()
    return nc


def kernel(inputs, weights, bias, profile=False, trace_kwargs=None):
    x = np.ascontiguousarray(inputs, dtype=np.float32)
    w = np.ascontiguousarray(weights, dtype=np.float32)

    # Input transform V = B^T d per (row, tile): [B, c, xi, 64*31] bf16
    xe = x[..., 0::2]   # [B, C, 64, 32]
    xo = x[..., 1::2]
    v = np.empty((B_FULL, C_IN, N_XI, H, N_T), dtype=np.float32)
    v[:, :, 0] = xe[..., :N_T] - xe[..., 1 : N_T + 1]
    v[:, :, 1] = xo[..., :N_T] + xe[..., 1 : N_T + 1]
    v[:, :, 2] = xe[..., 1 : N_T + 1] - xo[..., :N_T]
    v[:, :, 3] = xo[..., :N_T] - xo[..., 1 : N_T + 1]
    v_bf = np.ascontiguousarray(
        v.reshape(B_FULL, C_IN, N_XI, H * N_T)
    ).astype(ml_dtypes.bfloat16)

    # Weight transform Gw: [O, C, u, v] -> [c, half, xi, u, o_local] bf16
    g0 = w[..., 0]
    g1 = (w[..., 0] + w[..., 1] + w[..., 2]) * 0.5
    g2 = (w[..., 0] - w[..., 1] + w[..., 2]) * 0.5
    g3 = w[..., 2]
    gw = np.stack([g0, g1, g2, g3], axis=2)     # [O, C, xi, u]
    gw = gw.reshape(N_HALF, 128, C_IN, N_XI, KH).transpose(2, 0, 3, 4, 1)
    gw_bf = np.ascontiguousarray(gw).astype(ml_dtypes.bfloat16)

    bb = bias.astype(np.float32).reshape(N_HALF, 128).T   # [128, half]
    b_t = np.ascontiguousarray(
        np.concatenate([bb, -bb], axis=1)                 # [128, 4]
    )

    if "nc" not in _cached:
        _cached["nc"] = _build_nc()
    nc = _cached["nc"]

    in_maps = [
        {
            "v": v_bf[i * B_LOC : (i + 1) * B_LOC],
            "gw": gw_bf,
            "bias_in": b_t,
        }
        for i in range(N_CORES)
    ]
    res = run_bass_kernel_spmd(
        nc,
        in_maps,
        list(range(N_CORES)),
        trace=profile,
        **(trace_kwargs or {}),
    )
    _cached["last_result"] = res

    shards = []
    for i in range(N_CORES):
        y = res.results[i]["y"]  # [B_LOC, 2, 128, 2, 62*31] bf16
        y = np.asarray(y).astype(np.float32)
        y = y.reshape(B_LOC, C_OUT, 2, H_OUT, N_T)
        # [b, o, parity, i, t] -> [b, o, i, t, parity] -> [b, o, 62, 62]
        y = y.transpose(0, 1, 3, 4, 2).reshape(B_LOC, C_OUT, H_OUT, W_OUT)
        shards.append(y)
    return np.ascontiguousarray(np.concatenate(shards, axis=0), dtype=np.float32)
